# revision 6
# baseline (speedup 1.0000x reference)
"""Trainium2 Bass kernel for packed varlen (block-diagonal) decoder self-attention.

Problem: x[N=12288, C=768] packed tokens of B=16 sequences (cu_seqlens),
qkv proj (C -> 3C), H=12 heads x D=64 bidirectional attention within each
sequence, out proj (C -> C).

Sharding: whole sequences are distributed over 8 cores, 2 per core ("slot A"
holds one of the 8 longest, "slot B" one of the 8 shortest -> balanced).
All weights are replicated.  One SPMD program runs on all 8 cores; per-core
raggedness is handled with zero-padded K/V (pad keys score exactly 0 ->
exp = 1 -> subtract a per-core pad count from the softmax denominator).

Dataflow (all "transposed": channels on partitions, tokens on the free dim):
  xT[c,t] --(W stationary)--> qT,kT[oc,t] ; V[t,vc] via xT-stationary matmuls
  scores_T[k,q] = kT.T-chunk x qT  (two heads packed by row-tiling, K=64)
  P = exp(scale*s) on ACT (PSUM->SBUF), denominator = ones-matmul over P-accum
  out_T[hd,q] = V-chunk x P accumulated over key chunks (two heads col-tiled)
  y[t,oc] = out_T-stationary x Wproj (+bias), contiguous DMA out.

Matmuls use float32r (full-speed fp32, ~13-bit mantissa, fp32 accumulate).
"""

import sys
import os

sys.path.insert(0, "/opt/trn_rl_repo")

import numpy as np

C = 768
H = 12
D = 64
NHEADPAIR = 6
NCHUNK = 6  # 768 / 128
NCORES = 8
PARTN = 128

_cache = {}


# --------------------------------------------------------------------------- #
# BIR post-pass: the walrus build in this container rejects instructions with
# more than one semaphore wait ("Too many sync wait commands").  Hoist excess
# waits onto single-wait NoOps on the same engine (engines dispatch in order,
# so a preceding NoOp's wait gates everything after it).
# --------------------------------------------------------------------------- #
def _split_multiwait(nc, keep=1):
    import concourse.mybir as mybir

    for f in nc.m.functions:
        for b in f.blocks:
            out = []
            for inst in b.instructions:
                si = inst.sync_info
                if si is not None and si.on_wait and len(si.on_wait) > keep:
                    waits = list(si.on_wait)
                    extra, kept = waits[:-keep], waits[-keep:]
                    for k, w in enumerate(extra):
                        out.append(
                            mybir.InstNoOp(
                                name=f"{inst.name}-ws{k}",
                                sync_info=mybir.SyncInfo(on_wait=[w], on_update=[]),
                                bass_nofuse=True,
                                engine=inst.engine,
                            )
                        )
                    si.on_wait = kept
                    inst.sync_info = si
                out.append(inst)
            b.instructions = out


# --------------------------------------------------------------------------- #
# Kernel construction
# --------------------------------------------------------------------------- #
def _build(nb_a, nb_b, has_qkvb, has_projb):
    import concourse.bass as bass
    import concourse.mybir as mybir
    from concourse.tile import TileContext

    f32 = mybir.dt.float32
    f32r = mybir.dt.float32r
    bf16 = mybir.dt.bfloat16
    EXP = mybir.ActivationFunctionType.Exp
    IDENT = mybir.ActivationFunctionType.Identity

    nbs = [nb_a, nb_b] if nb_b > 0 else [nb_a]
    T = (nb_a + nb_b) * PARTN
    scale = float(D) ** -0.5

    nc = bass.Bass()
    xT = nc.dram_tensor("xt", (C, T), f32r, kind="ExternalInput")
    wqkv = nc.dram_tensor("wqkv", (C, 3 * C), f32r, kind="ExternalInput")
    wp = nc.dram_tensor("wp", (C, C), f32r, kind="ExternalInput")
    bqk = nc.dram_tensor("bqk", (PARTN, 2 * NCHUNK), f32, kind="ExternalInput")
    bv = nc.dram_tensor("bv", (1, C), f32r, kind="ExternalInput")
    bp = nc.dram_tensor("bp", (1, C), f32r, kind="ExternalInput")
    npad = nc.dram_tensor("npad", (PARTN, 2), f32, kind="ExternalInput")
    onesr = nc.dram_tensor("onesr", (1, PARTN), f32r, kind="ExternalInput")
    y = nc.dram_tensor("y", (T, C), f32, kind="ExternalOutput")

    with TileContext(nc) as tc:
        with (
            tc.tile_pool(name="const", bufs=1) as constp,
            tc.tile_pool(name="xtp", bufs=2, space="SBUF") as xtp,
            tc.tile_pool(name="qkp", bufs=1) as qkp,
            tc.tile_pool(name="vp", bufs=1) as vpool,
            tc.tile_pool(name="pp", bufs=2) as ppool,
            tc.tile_pool(name="accp", bufs=2) as accp,
            tc.tile_pool(name="rp", bufs=2) as rpool,
            tc.tile_pool(name="atp", bufs=1) as atp,
            tc.tile_pool(name="yp", bufs=2) as ypool,
            tc.tile_pool(name="ps_mm", bufs=2, space="PSUM") as ps_mm,
            tc.tile_pool(name="ps_sc", bufs=4, space="PSUM") as ps_sc,
            tc.tile_pool(name="ps_pv", bufs=2, space="PSUM") as ps_pv,
        ):
            # ---- resident constants ----
            w = []
            for c in range(NCHUNK):
                t = constp.tile([PARTN, 3 * C], f32r, name=f"w{c}", tag=f"w{c}")
                nc.sync.dma_start(t[:], wqkv[c * PARTN:(c + 1) * PARTN, :])
                w.append(t)
            wpt = []
            for c in range(NCHUNK):
                t = constp.tile([PARTN, C], f32r, name=f"wp{c}", tag=f"wp{c}")
                nc.sync.dma_start(t[:], wp[c * PARTN:(c + 1) * PARTN, :])
                wpt.append(t)
            onesb = constp.tile([PARTN, D], bf16, name="onesb", tag="onesb")
            nc.vector.memset(onesb[:], 1.0)
            if has_qkvb or has_projb:
                ones1 = constp.tile([1, PARTN], f32r, name="ones1", tag="ones1")
                nc.sync.dma_start(ones1[:], onesr[:])
            npad_sb = constp.tile([PARTN, 2], f32, name="npad_sb", tag="npad_sb")
            nc.sync.dma_start(npad_sb[:], npad[:])
            if has_qkvb:
                bqk_sb = constp.tile([PARTN, 2 * NCHUNK], f32, name="bqk_sb", tag="bqk_sb")
                nc.sync.dma_start(bqk_sb[:], bqk[:])
                bv_sb = constp.tile([1, C], f32r, name="bv_sb", tag="bv_sb")
                nc.sync.dma_start(bv_sb[:], bv[:])
            if has_projb:
                bp_sb = constp.tile([1, C], f32r, name="bp_sb", tag="bp_sb")
                nc.sync.dma_start(bp_sb[:], bp[:])

            t0 = 0
            for s, nb in enumerate(nbs):
                ts = nb * PARTN

                # ---- QKV projection for this slot ----
                qst = [
                    qkp.tile([PARTN, ts], f32r, name=f"q{s}_{j}", tag=f"qk{2*j}")
                    for j in range(NCHUNK)
                ]
                kst = [
                    qkp.tile([PARTN, ts], f32r, name=f"k{s}_{j}", tag=f"qk{2*j+1}")
                    for j in range(NCHUNK)
                ]
                vst = [
                    vpool.tile([PARTN, C], bf16, name=f"v{s}_{i}", tag=f"v{i}")
                    for i in range(nb)
                ]
                for tt in range(0, ts, 512):
                    tw = min(512, ts - tt)
                    xt = []
                    for c in range(NCHUNK):
                        t = xtp.tile([PARTN, tw], f32r, name=f"xt{c}", tag=f"xt{c}")
                        nc.sync.dma_start(
                            t[:], xT[c * PARTN:(c + 1) * PARTN, t0 + tt:t0 + tt + tw]
                        )
                        xt.append(t)
                    # Q (oc 0..5) and K (oc 6..11): W-chunk stationary, xT moving
                    for oc in range(2 * NCHUNK):
                        ps = ps_mm.tile([PARTN, tw], f32, name="psqk", tag="mm")
                        for c in range(NCHUNK):
                            nc.tensor.matmul(
                                ps[:],
                                w[c][:, oc * PARTN:(oc + 1) * PARTN],
                                xt[c][:],
                                start=(c == 0),
                                stop=(c == NCHUNK - 1),
                            )
                        dst = qst[oc] if oc < NCHUNK else kst[oc - NCHUNK]
                        if has_qkvb:
                            nc.scalar.activation(
                                dst[:, tt:tt + tw], ps[:], IDENT,
                                bias=bqk_sb[:, oc:oc + 1],
                            )
                        else:
                            nc.scalar.copy(dst[:, tt:tt + tw], ps[:])
                    # V: xT-chunk stationary, W moving -> untransposed [tok, vc]
                    for tk in range(tw // PARTN):
                        vten = vst[(tt + tk * PARTN) // PARTN]
                        for vc0, vw in ((0, 512), (512, 256)):
                            ps = ps_mm.tile([PARTN, vw], f32, name="psv", tag="mm")
                            for c in range(NCHUNK):
                                nc.tensor.matmul(
                                    ps[:],
                                    xt[c][:, tk * PARTN:(tk + 1) * PARTN],
                                    w[c][:, 2 * C + vc0:2 * C + vc0 + vw],
                                    start=(c == 0),
                                    stop=(c == NCHUNK - 1) if not has_qkvb else False,
                                )
                            if has_qkvb:
                                nc.tensor.matmul(
                                    ps[:],
                                    ones1[0:1, 0:PARTN],
                                    bv_sb[0:1, vc0:vc0 + vw],
                                    start=False,
                                    stop=True,
                                )
                            nc.vector.tensor_copy(vten[:, vc0:vc0 + vw], ps[:])

                # ---- attention + out-proj per query tile ----
                for q0 in range(0, ts, 512):
                    qw = min(512, ts - q0)
                    at = [
                        atp.tile([PARTN, qw], f32r, name=f"at{j}", tag=f"at{j}")
                        for j in range(NHEADPAIR)
                    ]
                    for j in range(NHEADPAIR):
                        pv = ps_pv.tile([PARTN, qw], f32, name="pv", tag="pv")
                        acc1 = accp.tile([PARTN, qw], bf16, name="acc1", tag="acc1")
                        acc2 = accp.tile([PARTN, qw], bf16, name="acc2", tag="acc2")
                        for c in range(nb):
                            s1 = ps_sc.tile([PARTN, qw], f32, name="s1", tag="sc")
                            s2 = ps_sc.tile([PARTN, qw], f32, name="s2", tag="sc")
                            nc.tensor.matmul(
                                s1[:],
                                kst[j][0:D, c * PARTN:(c + 1) * PARTN],
                                qst[j][0:D, q0:q0 + qw],
                                tile_position=(0, 0),
                            )
                            nc.tensor.matmul(
                                s2[:],
                                kst[j][D:PARTN, c * PARTN:(c + 1) * PARTN],
                                qst[j][D:PARTN, q0:q0 + qw],
                                tile_position=(D, 0),
                            )
                            p1 = ppool.tile([PARTN, qw], bf16, name="p1", tag="p1")
                            p2 = ppool.tile([PARTN, qw], bf16, name="p2", tag="p2")
                            nc.scalar.activation(p1[:], s1[:], EXP, scale=scale)
                            nc.scalar.activation(p2[:], s2[:], EXP, scale=scale)
                            if c == 0:
                                nc.vector.tensor_copy(acc1[:], p1[:])
                                nc.vector.tensor_copy(acc2[:], p2[:])
                            else:
                                nc.vector.tensor_add(acc1[:], acc1[:], p1[:])
                                nc.vector.tensor_add(acc2[:], acc2[:], p2[:])
                            vt = vst[c]
                            nc.tensor.matmul(
                                pv[0:D, :],
                                vt[:, j * PARTN:j * PARTN + D],
                                p1[:],
                                tile_position=(0, 0),
                                start=(c == 0),
                                stop=(c == nb - 1),
                            )
                            nc.tensor.matmul(
                                pv[D:PARTN, :],
                                vt[:, j * PARTN + D:(j + 1) * PARTN],
                                p2[:],
                                tile_position=(0, D),
                                start=(c == 0),
                                stop=(c == nb - 1),
                            )
                        # denominators, broadcast across partitions by a
                        # ones[128,64]-stationary matmul (every output row is
                        # the key-axis sum)
                        dsum = ps_mm.tile([PARTN, qw], f32, name="dsum", tag="mm")
                        nc.tensor.matmul(
                            dsum[0:D, :], onesb[:, 0:D], acc1[:], tile_position=(0, 0)
                        )
                        nc.tensor.matmul(
                            dsum[D:PARTN, :], onesb[:, 0:D], acc2[:], tile_position=(0, D)
                        )
                        rb = rpool.tile([PARTN, qw], f32, name="rb", tag="rb")
                        nc.vector.tensor_scalar_add(rb[:], dsum[:], npad_sb[:, s:s + 1])
                        nc.vector.reciprocal(rb[:], rb[:])
                        nc.vector.tensor_mul(at[j][:], pv[:], rb[:])

                    # out-projection for this query tile
                    for tk in range(qw // PARTN):
                        ysb = ypool.tile([PARTN, C], f32, name="ysb", tag="ysb")
                        for oc0, ow in ((0, 512), (512, 256)):
                            ps = ps_mm.tile([PARTN, ow], f32, name="psy", tag="mm")
                            for c in range(NCHUNK):
                                nc.tensor.matmul(
                                    ps[:],
                                    at[c][:, tk * PARTN:(tk + 1) * PARTN],
                                    wpt[c][:, oc0:oc0 + ow],
                                    start=(c == 0),
                                    stop=(c == NCHUNK - 1) if not has_projb else False,
                                )
                            if has_projb:
                                nc.tensor.matmul(
                                    ps[:],
                                    ones1[0:1, 0:PARTN],
                                    bp_sb[0:1, oc0:oc0 + ow],
                                    start=False,
                                    stop=True,
                                )
                            nc.scalar.copy(ysb[:, oc0:oc0 + ow], ps[:])
                        nc.sync.dma_start(
                            y[t0 + q0 + tk * PARTN:t0 + q0 + (tk + 1) * PARTN, :],
                            ysb[:],
                        )
                t0 += ts

    _split_multiwait(nc)
    return nc


# --------------------------------------------------------------------------- #
# Cached compile + SPMD execution (axon PJRT path, mirrors run_bass_via_pjrt
# but keeps the jitted executable so repeated calls don't recompile)
# --------------------------------------------------------------------------- #
def _get_runner(key, nb_a, nb_b, has_qkvb, has_projb):
    if key in _cache:
        return _cache[key]

    import jax
    import concourse.mybir as mybir
    from concourse import bass2jax
    from jax.sharding import Mesh, PartitionSpec
    from jax.experimental.shard_map import shard_map

    nc = _build(nb_a, nb_b, has_qkvb, has_projb)
    bass2jax.install_neuronx_cc_hook()

    partition_name = nc.partition_id_tensor.name if nc.partition_id_tensor else None
    in_names = []
    out_names = []
    out_avals = []
    zero_outs = []
    for alloc in nc.m.functions[0].allocations:
        if not isinstance(alloc, mybir.MemoryLocationSet):
            continue
        name = alloc.memorylocations[0].name
        if alloc.kind == "ExternalInput":
            if name != partition_name:
                in_names.append(name)
        elif alloc.kind == "ExternalOutput":
            out_names.append(name)
            shape = tuple(alloc.tensor_shape)
            dtype = mybir.dt.np(alloc.dtype)
            out_avals.append(jax.core.ShapedArray(shape, dtype))
            zero_outs.append(np.zeros(shape, dtype))
    n_params = len(in_names)
    n_outs = len(out_avals)
    all_names = in_names + out_names
    if partition_name is not None:
        all_names = all_names + [partition_name]
    donate = tuple(range(n_params, n_params + n_outs))

    def _body(*args):
        operands = list(args)
        if partition_name is not None:
            operands.append(bass2jax.partition_id_tensor())
        outs = bass2jax._bass_exec_p.bind(
            *operands,
            out_avals=tuple(out_avals),
            in_names=tuple(all_names),
            out_names=tuple(out_names),
            lowering_input_output_aliases=(),
            sim_require_finite=True,
            sim_require_nnan=True,
            nc=nc,
        )
        return tuple(outs)

    devices = jax.devices()[:NCORES]
    mesh = Mesh(np.asarray(devices), ("core",))
    sharded = jax.jit(
        shard_map(
            _body,
            mesh=mesh,
            in_specs=(PartitionSpec("core"),) * (n_params + n_outs),
            out_specs=(PartitionSpec("core"),) * n_outs,
            check_rep=False,
        ),
        donate_argnums=donate,
        keep_unused=True,
    )

    def run(in_maps):
        concat_in = [
            np.concatenate([np.asarray(m[name]) for m in in_maps], axis=0)
            for name in in_names
        ]
        concat_zeros = [
            np.zeros((NCORES * z.shape[0], *z.shape[1:]), z.dtype) for z in zero_outs
        ]
        out_arrs = sharded(*concat_in, *concat_zeros)
        return [
            {
                name: np.asarray(out_arrs[i]).reshape(NCORES, *out_avals[i].shape)[c]
                for i, name in enumerate(out_names)
            }
            for c in range(NCORES)
        ]

    _cache[key] = (run, nc)
    return _cache[key]


# --------------------------------------------------------------------------- #
# Host-side sharding / unsharding
# --------------------------------------------------------------------------- #
def kernel(x, qkv_w, qkv_b, proj_w, proj_b, cu_seqlens, max_seqlen):
    x = np.ascontiguousarray(np.asarray(x, dtype=np.float32))
    qkv_w = np.ascontiguousarray(np.asarray(qkv_w, dtype=np.float32))
    qkv_b = np.asarray(qkv_b, dtype=np.float32)
    proj_w = np.ascontiguousarray(np.asarray(proj_w, dtype=np.float32))
    proj_b = np.asarray(proj_b, dtype=np.float32)
    cu = np.asarray(cu_seqlens).astype(np.int64)
    L = int(np.asarray(max_seqlen))
    N = x.shape[0]
    B = cu.shape[0] - 1

    idx = np.arange(N)
    bid = np.searchsorted(cu[1:], idx, side="right")
    pos = idx - cu[np.minimum(bid, B)]

    # valid (participating) tokens per sequence: contiguous positions 0..Lr-1
    starts = np.zeros(B, np.int64)
    Lr = np.zeros(B, np.int64)
    for i in range(B):
        m = (bid == i) & (pos >= 0) & (pos < L)
        if m.any():
            ii = idx[m]
            starts[i] = ii[0]
            Lr[i] = ii.shape[0]

    nb = (Lr + PARTN - 1) // PARTN  # 128-blocks per sequence
    order = np.argsort(-nb, kind="stable")
    slot_a = order[:NCORES]
    slot_b = order[NCORES:2 * NCORES][::-1]
    nb_a = int(nb[slot_a].max()) if len(slot_a) else 0
    nb_b = int(nb[slot_b].max()) if len(slot_b) else 0

    if nb_a == 0:
        # every sequence is empty: reference output is proj_b everywhere
        return np.broadcast_to(proj_b, (N, C)).copy().astype(np.float32)

    has_qkvb = bool(np.any(qkv_b))
    has_projb = bool(np.any(proj_b))
    T = (nb_a + nb_b) * PARTN

    run, _ = _get_runner((N, T, nb_a, nb_b, has_qkvb, has_projb),
                         nb_a, nb_b, has_qkvb, has_projb)

    # per-core inputs
    bqk_in = qkv_b[:2 * C].reshape(2 * NCHUNK, PARTN).T.copy()  # [128, 12]
    bv_in = qkv_b[2 * C:].reshape(1, C)
    bp_in = proj_b.reshape(1, C)
    seq_core = {}
    seq_off = {}
    in_maps = []
    for cidx in range(NCORES):
        xc = np.zeros((T, C), np.float32)
        npad_c = np.zeros((PARTN, 2), np.float32)
        for s, (seq, nbs) in enumerate(((slot_a[cidx], nb_a), (slot_b[cidx], nb_b))):
            if nbs == 0:
                continue
            off = 0 if s == 0 else nb_a * PARTN
            lr = int(Lr[seq])
            if lr > 0:
                st = int(starts[seq])
                xc[off:off + lr] = x[st:st + lr]
            npad_c[:, s] = -(nbs * PARTN - lr)
            seq_core[int(seq)] = cidx
            seq_off[int(seq)] = off
        in_maps.append(
            {
                "xt": np.ascontiguousarray(xc.T),
                "wqkv": qkv_w,
                "wp": proj_w,
                "bqk": bqk_in,
                "bv": bv_in,
                "bp": bp_in,
                "npad": npad_c,
                "onesr": np.ones((1, PARTN), np.float32),
            }
        )

    results = run(in_maps)

    # unshard: replicate the reference's clamped-gather semantics
    y_full = np.empty((N, C), np.float32)
    bid_c = np.minimum(bid, B - 1)
    pos_c = np.clip(pos, 0, L - 1)
    for i in range(B):
        rows = bid_c == i
        if not rows.any():
            continue
        if Lr[i] == 0:
            y_full[rows] = proj_b
        else:
            yc = results[seq_core[i]]["y"]
            y_full[rows] = yc[seq_off[i] + pos_c[rows]]
    return y_full
